# revision 5
# baseline (speedup 1.0000x reference)
"""Trainium2 Bass kernel for nn_AFM (attention-modulated 3x3 conv).

Math (per batch):
    ratio = MLP_a2(mean_hw(x))                       # [9]
    bias3 = MLP_a3(mean_hw(x))                       # [64]
    y[m,p] = sum_{c,t} W[m,c,t] * x[c, p+delta_t] * (atw1[t,p]*ratio[t]) + bias3[m]

Strategy: data-parallel over batch (8 cores, 1 batch each, no collectives).
Per core, fp16 compute:
  - attention (with conv zero-pad validity baked in as zeros, guard columns)
    ships as an f16 input `ag`; x ships as guarded f16 `xg`.
  - ratio is folded into the conv weights on device (so atw1 is used raw).
  - taps are processed in pairs sharing one 128-row contraction:
      R_pair[(c,t), q] = x[c, q+u_t] * ag[t, q - o_pair]
    built by DVE f16 tensor_tensor (2x mode), with the per-pair attention
    replica tiles produced by DRAM->SBUF broadcast DMA (partition step 0).
  - 5 PSUM-accumulated f16 matmuls per 512-pixel tile compute y directly;
    ScalarE evacuates PSUM with the a3-bias add fused.
"""

import numpy as np
from contextlib import ExitStack

import concourse.bass as bass
import concourse.tile as tile
from concourse import bacc, mybir
from concourse.bass_utils import run_bass_kernel_spmd

# permuted tap order: rows are taps [0,2, 3,5, 6,8, 1,4, 7] so that each
# matmul group's two taps sit on adjacent rows of `ag`
PERM = [0, 2, 3, 5, 6, 8, 1, 4, 7]
# groups: (row0, row1|None, o = rhs pixel offset, u = upper-half x shift)
GROUPS = [
    (0, 1, -129, 2),
    (2, 3, -1, 2),
    (4, 5, 127, 2),
    (6, 7, -128, 128),
    (8, None, 128, 0),
]
HH = 128
WW = 128
P = HH * WW           # 16384 pixels
CIN = 64
COUT = 64
GUARD = 264           # zero guard columns on xg/ag (>= 132 + 129)
GL = 132              # per-chunk halo for rhs offsets (|o| <= 129)
CH = 2048             # pixels per chunk
NCH = P // CH
WCH = CH + 2 * GL     # 2312 columns per R/ar chunk tile
WX = P + 2 * GUARD    # 16912

_CACHE = {}


def _build_nc():
    f32, f16 = mybir.dt.float32, mybir.dt.float16
    AF = mybir.ActivationFunctionType
    OP = mybir.AluOpType

    nc = bacc.Bacc("TRN2", target_bir_lowering=False, debug=False,
                   enable_asserts=True, num_devices=8)
    xg = nc.dram_tensor("xg", [CIN, WX], f16, kind="ExternalInput").ap()
    ag = nc.dram_tensor("ag", [9, WX], f16, kind="ExternalInput").ap()
    wl = nc.dram_tensor("wl", [5, 128, COUT], f32, kind="ExternalInput").ap()
    a2w1T = nc.dram_tensor("a2w1T", [CIN, 9], f32, kind="ExternalInput").ap()
    a2b1 = nc.dram_tensor("a2b1", [9, 1], f32, kind="ExternalInput").ap()
    a2w2r = nc.dram_tensor("a2w2r", [9, 5 * 128], f32, kind="ExternalInput").ap()
    a2b2r = nc.dram_tensor("a2b2r", [128, 5], f32, kind="ExternalInput").ap()
    a3w1T = nc.dram_tensor("a3w1T", [CIN, COUT], f32, kind="ExternalInput").ap()
    a3b1 = nc.dram_tensor("a3b1", [COUT, 1], f32, kind="ExternalInput").ap()
    a3w2T = nc.dram_tensor("a3w2T", [COUT, COUT], f32, kind="ExternalInput").ap()
    a3b2 = nc.dram_tensor("a3b2", [COUT, 1], f32, kind="ExternalInput").ap()
    y = nc.dram_tensor("y", [COUT, P], f32, kind="ExternalOutput").ap()

    with tile.TileContext(nc) as tc:
        with ExitStack() as ctx:
            sing = ctx.enter_context(tc.tile_pool(name="sing", bufs=1))
            arp = ctx.enter_context(tc.tile_pool(name="arp", bufs=2))
            rp = ctx.enter_context(tc.tile_pool(name="rp", bufs=2))
            yp = ctx.enter_context(tc.tile_pool(name="yp", bufs=2))
            xin = ctx.enter_context(tc.tile_pool(name="xin", bufs=2))
            psy = ctx.enter_context(tc.tile_pool(name="psy", bufs=2, space="PSUM"))
            psm = ctx.enter_context(tc.tile_pool(name="psm", bufs=2, space="PSUM"))

            # x (f16, guarded) -> lower half of XXL; upper half = x shifted +2
            XXL = sing.tile([128, WX], f16)
            nc.sync.dma_start(out=XXL[0:64, :], in_=xg)
            nc.vector.tensor_copy(out=XXL[64:128, 0:WX - 2], in_=XXL[0:64, 2:WX])
            nc.vector.memset(XXL[64:128, WX - 2:WX], 0)

            # per-channel sums for g = mean(x): ScalarE copy with accumulate
            gparts = sing.tile([CIN, NCH], f32)
            trash = sing.tile([CIN, CH], f16)
            for k in range(NCH):
                nc.scalar.activation(
                    out=trash[:], in_=XXL[0:64, GUARD + k * CH: GUARD + (k + 1) * CH],
                    func=AF.Copy, accum_out=gparts[:, k:k + 1])
            gsum = sing.tile([CIN, 1], f32)
            nc.vector.tensor_reduce(gsum[:], gparts[:], axis=mybir.AxisListType.X,
                                    op=OP.add)

            # small-weight loads
            w_a2w1T = sing.tile([CIN, 9], f32)
            nc.sync.dma_start(out=w_a2w1T, in_=a2w1T)
            w_a2b1 = sing.tile([9, 1], f32)
            nc.sync.dma_start(out=w_a2b1, in_=a2b1)
            w_a2w2r = sing.tile([9, 5 * 128], f32)
            nc.sync.dma_start(out=w_a2w2r, in_=a2w2r)
            w_a2b2r = sing.tile([128, 5], f32)
            nc.sync.dma_start(out=w_a2b2r, in_=a2b2r)
            w_a3w1T = sing.tile([CIN, COUT], f32)
            nc.sync.dma_start(out=w_a3w1T, in_=a3w1T)
            w_a3b1 = sing.tile([COUT, 1], f32)
            nc.sync.dma_start(out=w_a3b1, in_=a3b1)
            w_a3w2T = sing.tile([COUT, COUT], f32)
            nc.sync.dma_start(out=w_a3w2T, in_=a3w2T)
            w_a3b2 = sing.tile([COUT, 1], f32)
            nc.sync.dma_start(out=w_a3b2, in_=a3b2)
            wlt = []
            for g in range(5):
                t = sing.tile([128, COUT], f32, tag=f"wl{g}")
                nc.sync.dma_start(out=t, in_=wl[g])
                wlt.append(t)

            # attention2: h1 = relu(g @ a2w1.T / P + b1); ratio (replicated per
            # group via host-widened a2w2 columns)
            h1ps = psm.tile([9, 1], f32, tag="mlp")
            nc.tensor.matmul(h1ps[:], w_a2w1T[:], gsum[:], start=True, stop=True)
            h1r = sing.tile([9, 1], f32)
            nc.scalar.activation(out=h1r[:], in_=h1ps[:], func=AF.Relu,
                                 bias=w_a2b1[:], scale=1.0 / P)
            rrep = []
            for g in range(5):
                rps = psm.tile([128, 1], f32, tag="mlp")
                nc.tensor.matmul(rps[:], w_a2w2r[:, g * 128:(g + 1) * 128], h1r[:],
                                 start=True, stop=True)
                rr = sing.tile([128, 1], f32, tag=f"rr{g}")
                nc.scalar.activation(out=rr[:], in_=rps[:], func=AF.Identity,
                                     bias=w_a2b2r[:, g:g + 1], scale=1.0)
                rrep.append(rr)

            # attention3 bias: bias3 = relu(g @ a3w1.T / P + b1) @ a3w2.T + b2
            h3ps = psm.tile([COUT, 1], f32, tag="mlp")
            nc.tensor.matmul(h3ps[:], w_a3w1T[:], gsum[:], start=True, stop=True)
            h3r = sing.tile([COUT, 1], f32)
            nc.scalar.activation(out=h3r[:], in_=h3ps[:], func=AF.Relu,
                                 bias=w_a3b1[:], scale=1.0 / P)
            b3ps = psm.tile([COUT, 1], f32, tag="mlp")
            nc.tensor.matmul(b3ps[:], w_a3w2T[:], h3r[:], start=True, stop=True)
            bias3 = sing.tile([COUT, 1], f32)
            nc.scalar.activation(out=bias3[:], in_=b3ps[:], func=AF.Identity,
                                 bias=w_a3b2[:], scale=1.0)

            # fold ratio into conv weights: wf_g = wl_g * rrep_g  (f16)
            wf = []
            for g, (r0, r1, o, u) in enumerate(GROUPS):
                parts = 128 if r1 is not None else 64
                t = sing.tile([parts, COUT], f16, tag=f"wf{g}")
                nc.vector.tensor_scalar(t[:], wlt[g][0:parts, :], rrep[g][0:parts, :],
                                        None, OP.mult)
                wf.append(t)

            # main pixel-chunk loop
            for k in range(NCH):
                col0 = k * CH + GUARD - GL  # XXL/ag column of q0 = k*CH - GL
                # attention replica tiles (DRAM->SBUF broadcast DMA)
                arts = []
                for g, (r0, r1, o, u) in enumerate(GROUPS):
                    s0 = col0 - o

                    def bcast_row(r):
                        row = ag[r:r + 1, s0:s0 + WCH]
                        return bass.AP(tensor=row.tensor, offset=row.offset,
                                       ap=[[0, 64], list(row.ap)[-1]])

                    if g == 3:
                        # split tiles: TensorTensor needs equal input base
                        # partitions, and this group's two halves multiply
                        # the same base-0 x window
                        a0 = arp.tile([64, WCH], f16, tag="ar3a")
                        a1 = arp.tile([64, WCH], f16, tag="ar3b")
                        nc.sync.dma_start(out=a0[:], in_=bcast_row(r0))
                        nc.sync.dma_start(out=a1[:], in_=bcast_row(r1))
                        arts.append((a0, a1))
                        continue
                    parts = 128 if r1 is not None else 64
                    art = arp.tile([parts, WCH], f16, tag=f"ar{g}")
                    nc.sync.dma_start(out=art[0:64, :], in_=bcast_row(r0))
                    if r1 is not None:
                        nc.sync.dma_start(out=art[64:128, :], in_=bcast_row(r1))
                    arts.append(art)

                # R tiles: f16 tensor_tensor multiplies (2x mode)
                rts = []
                for g, (r0, r1, o, u) in enumerate(GROUPS):
                    parts = 128 if r1 is not None else 64
                    rt = rp.tile([parts, WCH], f16, tag=f"r{g}")
                    if g < 3:
                        # pair via XXL (upper half pre-shifted by +2)
                        nc.vector.tensor_mul(rt[:], XXL[:, col0:col0 + WCH],
                                             arts[g][:])
                    elif g == 3:
                        nc.vector.tensor_mul(rt[0:64, :],
                                             XXL[0:64, col0:col0 + WCH],
                                             arts[g][0][:])
                        nc.vector.tensor_mul(rt[64:128, :],
                                             XXL[0:64, col0 + 128:col0 + 128 + WCH],
                                             arts[g][1][:])
                    else:
                        nc.vector.tensor_mul(rt[:], XXL[0:64, col0:col0 + WCH],
                                             arts[g][:])
                    rts.append(rt)

                # matmuls: two 1024-pixel PSUM tiles per chunk, 512-col slices
                for half in range(2):
                    yps = psy.tile([COUT, 1024], f32)
                    for g, (r0, r1, o, u) in enumerate(GROUPS):
                        for s in range(2):
                            c0 = (o + GL) + half * 1024 + s * 512
                            nc.tensor.matmul(yps[:, s * 512:(s + 1) * 512],
                                             wf[g][:], rts[g][:, c0:c0 + 512],
                                             start=(g == 0), stop=(g == 4))
                    ysb = yp.tile([COUT, 1024], f32)
                    nc.scalar.activation(out=ysb[:], in_=yps[:], func=AF.Identity,
                                         bias=bias3[:], scale=1.0)
                    nc.sync.dma_start(
                        out=y[:, k * CH + half * 1024: k * CH + (half + 1) * 1024],
                        in_=ysb[:])
    nc.compile()
    return nc


def _host_prep(x, atw1, weight, a2_w1, a2_b1, a2_w2, a2_b2, a3_w1, a3_b1,
               a3_w2, a3_b2):
    """Build per-core input maps (host-side layout/dtype prep only)."""
    b = x.shape[0]
    f16 = np.float16

    # validity mask per permuted row: conv zero-padding baked into attention
    hh = np.arange(P) // WW
    ww = np.arange(P) % WW
    masks = np.empty((9, P), np.bool_)
    for r, t in enumerate(PERM):
        kh, kw = t // 3, t % 3
        masks[r] = ((hh + kh - 1 >= 0) & (hh + kh - 1 < HH) &
                    (ww + kw - 1 >= 0) & (ww + kw - 1 < WW))

    # conv weight groups, transposed to [c, m]
    wlh = np.zeros((5, 128, COUT), np.float32)
    for g, (r0, r1, o, u) in enumerate(GROUPS):
        t0 = PERM[r0]
        wlh[g, 0:64] = weight[:, :, t0 // 3, t0 % 3].T
        if r1 is not None:
            t1 = PERM[r1]
            wlh[g, 64:128] = weight[:, :, t1 // 3, t1 % 3].T

    # a2_w2 widened so the ratio matmul directly produces per-group
    # [t0 x64 | t1 x64] replicated layouts
    a2w2r = np.zeros((9, 5 * 128), np.float32)
    a2b2r = np.zeros((128, 5), np.float32)
    for g, (r0, r1, o, u) in enumerate(GROUPS):
        a2w2r[:, g * 128: g * 128 + 64] = a2_w2[PERM[r0]][:, None]
        a2b2r[0:64, g] = a2_b2[PERM[r0]]
        if r1 is not None:
            a2w2r[:, g * 128 + 64: (g + 1) * 128] = a2_w2[PERM[r1]][:, None]
            a2b2r[64:128, g] = a2_b2[PERM[r1]]

    common = {
        "wl": np.ascontiguousarray(wlh),
        "a2w1T": np.ascontiguousarray(a2_w1.T.astype(np.float32)),
        "a2b1": np.ascontiguousarray(a2_b1[:, None].astype(np.float32)),
        "a2w2r": a2w2r,
        "a2b2r": a2b2r,
        "a3w1T": np.ascontiguousarray(a3_w1.T.astype(np.float32)),
        "a3b1": np.ascontiguousarray(a3_b1[:, None].astype(np.float32)),
        "a3w2T": np.ascontiguousarray(a3_w2.T.astype(np.float32)),
        "a3b2": np.ascontiguousarray(a3_b2[:, None].astype(np.float32)),
    }

    in_maps = []
    for i in range(b):
        xr = x[i].reshape(CIN, P)
        xgv = np.zeros((CIN, WX), f16)
        xgv[:, GUARD:GUARD + P] = xr.astype(f16)
        at = atw1[i].reshape(9, P)[PERM]
        agv = np.zeros((9, WX), f16)
        agv[:, GUARD:GUARD + P] = np.where(masks, at, 0.0).astype(f16)
        in_maps.append({"xg": xgv, "ag": agv, **common})
    return in_maps


def kernel(**inputs):
    x = np.asarray(inputs["x"], np.float32)
    in_maps = _host_prep(
        x, np.asarray(inputs["atw1"], np.float32),
        np.asarray(inputs["weight"], np.float32),
        np.asarray(inputs["a2_w1"], np.float32),
        np.asarray(inputs["a2_b1"], np.float32),
        np.asarray(inputs["a2_w2"], np.float32),
        np.asarray(inputs["a2_b2"], np.float32),
        np.asarray(inputs["a3_w1"], np.float32),
        np.asarray(inputs["a3_b1"], np.float32),
        np.asarray(inputs["a3_w2"], np.float32),
        np.asarray(inputs["a3_b2"], np.float32),
    )
    if "nc" not in _CACHE:
        _CACHE["nc"] = _build_nc()
    nc = _CACHE["nc"]
    res = run_bass_kernel_spmd(nc, in_maps, core_ids=list(range(8)))
    out = np.stack([res.results[i]["y"].reshape(COUT, HH, WW)
                    for i in range(len(in_maps))])
    return out.astype(np.float32)


# revision 20
# speedup vs baseline: 1.5833x; 1.5833x over previous
"""Trainium2 Bass kernel for nn_AFM (attention-modulated 3x3 conv).

Math (per batch):
    ratio = MLP_a2(mean_hw(x))                       # [9]
    bias3 = MLP_a3(mean_hw(x))                       # [64]
    y[m,p] = sum_{c,t} W[m,c,t] * x[c, p+delta_t] * (atw1[t,p]*ratio[t]) + bias3[m]

Strategy: data-parallel over batch (8 cores, 1 batch each, no collectives).
Per core, fp16 compute:
  - attention (with conv zero-pad validity baked in as zeros, guard columns)
    ships as an f16 input `ag`; x ships as guarded f16 `xg`.
  - ratio is folded into the conv weights on device (so atw1 is used raw).
  - taps are processed in pairs sharing one 128-row contraction:
      R_pair[(c,t), q] = x[c, q+u_t] * ag[t, q - o_pair]
    built by DVE f16 tensor_tensor (2x mode), with the per-pair attention
    replica tiles produced by DRAM->SBUF broadcast DMA (partition step 0),
    spread round-robin over issue engines to use multiple DMA queues.
  - 5 PSUM-accumulated f16 matmuls per 512-pixel tile compute y directly;
    ScalarE evacuates PSUM with the a3-bias add fused.
"""

import numpy as np
from contextlib import ExitStack

import concourse.bass as bass
import concourse.tile as tile
from concourse import bacc, mybir
from concourse.bass_utils import run_bass_kernel_spmd

# permuted tap order: rows are taps [0,2, 3,5, 6,8, 1,4, 7] so that each
# matmul group's two taps sit on adjacent rows of `ag`
PERM = [0, 2, 3, 5, 6, 8, 1, 4, 7]
# groups: (row0, row1|None, o = rhs pixel offset, u = upper-half x shift)
GROUPS = [
    (0, 1, -129, 2),
    (2, 3, -1, 2),
    (4, 5, 127, 2),
    (6, 7, -128, 128),
    (8, None, 128, 0),
]
HH = 128
WW = 128
P = HH * WW           # 16384 pixels
CIN = 64
COUT = 64
GUARD = 264           # zero guard columns on xg/ag (>= 132 + 129)
GL = 132              # per-chunk halo for rhs offsets (|o| <= 129)
CH = 2048             # pixels per chunk
NCH = P // CH
WCH = CH + 2 * GL     # 2312 columns per R/ar chunk tile
WX = P + 2 * GUARD    # 16912
AR_VIA_PE = ()    # pair-groups whose attention replicas come from a
                      # PE broadcast-matmul + ScalarE evac instead of DMA

# packed small-weight blob layout (f32, [128, BLOB_W]); columns:
#   wl: 5 groups x 64      -> 0..320     (rows 0..127)
#   a2w1T [64,9]           -> 320..329   (rows 0..63)
#   a2b1 [9,1]             -> 329..330   (rows 0..8)
#   a2w2r [9,640]          -> 330..970   (rows 0..8)
#   a2b2r [128,5]          -> 970..975
#   a3w1T [64,64]          -> 975..1039  (rows 0..63)
#   a3b1 [64,1]            -> 1039..1040
#   a3w2T [64,64]          -> 1040..1104
#   a3b2 [64,1]            -> 1104..1105
BLOB_W = 1105
C_WL, C_A2W1, C_A2B1, C_A2W2R, C_A2B2R = 0, 320, 329, 330, 970
C_A3W1, C_A3B1, C_A3W2, C_A3B2 = 975, 1039, 1040, 1104

_CACHE = {}


def _build_nc():
    f32, f16 = mybir.dt.float32, mybir.dt.float16
    AF = mybir.ActivationFunctionType
    OP = mybir.AluOpType

    nc = bacc.Bacc("TRN2", target_bir_lowering=False, debug=False,
                   enable_asserts=True, num_devices=8)
    xg = nc.dram_tensor("xg", [CIN, WX], f16, kind="ExternalInput").ap()
    ag = nc.dram_tensor("ag", [9, WX], f16, kind="ExternalInput").ap()
    wb = nc.dram_tensor("wb", [128, BLOB_W], f32, kind="ExternalInput").ap()
    if AR_VIA_PE:
        sg = nc.dram_tensor("sg", [34, 128], mybir.dt.float16,
                            kind="ExternalInput").ap()
    y = nc.dram_tensor("y", [COUT, P], f16, kind="ExternalOutput").ap()

    # round-robin DMA issue engines (separate queues)
    def dq(i):
        return [nc.sync, nc.scalar][i % 2]

    with tile.TileContext(nc) as tc:
        with ExitStack() as ctx:
            sing = ctx.enter_context(tc.tile_pool(name="sing", bufs=1))
            arp = ctx.enter_context(tc.tile_pool(name="arp", bufs=2))
            rp = ctx.enter_context(tc.tile_pool(name="rp", bufs=2))
            yp = ctx.enter_context(tc.tile_pool(name="yp", bufs=2))
            trp = ctx.enter_context(tc.tile_pool(name="trp", bufs=2))
            psy = ctx.enter_context(tc.tile_pool(name="psy", bufs=2, space="PSUM"))
            psm = ctx.enter_context(tc.tile_pool(name="psm", bufs=2, space="PSUM"))
            psb = ctx.enter_context(tc.tile_pool(name="psb", bufs=2, space="PSUM"))

            # small-weight blob
            wbt = sing.tile([128, BLOB_W], f32)
            nc.sync.dma_start(out=wbt, in_=wb)

            # attention rows in SBUF (PE broadcast source): one tile, each
            # PE-pair's 2 rows at a legal matmul base partition (0, 32)
            atg, sel = {}, {}
            if AR_VIA_PE:
                atgt = sing.tile([34, WX], f16)
                selt = sing.tile([34, 128], f16)
                nc.sync.dma_start(out=selt, in_=sg)
                for j, g in enumerate(AR_VIA_PE):
                    r0 = GROUPS[g][0]
                    nc.scalar.dma_start(out=atgt[32 * j:32 * j + 2, :],
                                        in_=ag[r0:r0 + 2, :])
                    atg[g] = atgt[32 * j:32 * j + 2, :]
                    sel[g] = selt[32 * j:32 * j + 2, :]

            # x (f16, guarded) -> lower half of XXL, loaded in chunks; the
            # g-mean accumulation runs per chunk, alternating ScalarE (f16
            # copy with accum_out) and VectorE (reduce); the +2-shifted upper
            # half is built in per-chunk pieces so early chunks unblock fast
            XXL = sing.tile([128, WX], f16)
            gparts = sing.tile([CIN, NCH], f32)
            nc.scalar.dma_start(out=XXL[0:64, 0:GUARD], in_=xg[:, 0:GUARD])
            nc.gpsimd.dma_start(out=XXL[0:64, GUARD + P:], in_=xg[:, GUARD + P:])
            for k in range(NCH):
                c0 = GUARD + k * CH
                dq(k).dma_start(out=XXL[0:64, c0:c0 + CH], in_=xg[:, c0:c0 + CH])
                if k % 2 == 0:
                    trash = trp.tile([CIN, CH], f16, tag="trash")
                    nc.scalar.activation(out=trash[:], in_=XXL[0:64, c0:c0 + CH],
                                         func=AF.Copy,
                                         accum_out=gparts[:, k:k + 1])
                else:
                    nc.vector.tensor_reduce(gparts[:, k:k + 1],
                                            XXL[0:64, c0:c0 + CH],
                                            axis=mybir.AxisListType.X, op=OP.add)
                if k > 0:
                    w0 = (k - 1) * CH + GUARD - GL
                    nc.vector.tensor_copy(out=XXL[64:128, w0:w0 + WCH],
                                          in_=XXL[0:64, w0 + 2:w0 + 2 + WCH])
            w0 = (NCH - 1) * CH + GUARD - GL
            nc.vector.tensor_copy(out=XXL[64:128, w0:w0 + WCH],
                                  in_=XXL[0:64, w0 + 2:w0 + 2 + WCH])

            # full-width replica of the lone tap's attention row, built once
            # (emitted after the x loads so they win queue priority)
            r8 = GROUPS[4][0]
            row8 = ag[r8:r8 + 1, :]
            ar8F = sing.tile([64, WX], f16)
            nc.sync.dma_start(out=ar8F[:], in_=bass.AP(
                tensor=row8.tensor, offset=row8.offset,
                ap=[[0, 64], list(row8.ap)[-1]]))

            gsum = sing.tile([CIN, 1], f32)
            nc.vector.tensor_reduce(gsum[:], gparts[:], axis=mybir.AxisListType.X,
                                    op=OP.add)

            # attention2: h1 = relu(g @ a2w1.T / P + b1); ratio replicated per
            # group via host-widened a2w2 columns
            h1ps = psm.tile([9, 1], f32, tag="mlp")
            nc.tensor.matmul(h1ps[:], wbt[0:64, C_A2W1:C_A2W1 + 9], gsum[:],
                             start=True, stop=True)
            h1r = sing.tile([9, 1], f32)
            nc.scalar.activation(out=h1r[:], in_=h1ps[:], func=AF.Relu,
                                 bias=wbt[0:9, C_A2B1:C_A2B1 + 1], scale=1.0 / P)
            rrep = []
            for g in range(5):
                rps = psm.tile([128, 1], f32, tag="mlp")
                nc.tensor.matmul(
                    rps[:], wbt[0:9, C_A2W2R + g * 128:C_A2W2R + (g + 1) * 128],
                    h1r[:], start=True, stop=True)
                rr = sing.tile([128, 1], f32, tag=f"rr{g}")
                nc.scalar.activation(out=rr[:], in_=rps[:], func=AF.Identity,
                                     bias=wbt[:, C_A2B2R + g:C_A2B2R + g + 1],
                                     scale=1.0)
                rrep.append(rr)

            # attention3 bias: bias3 = relu(g @ a3w1.T / P + b1) @ a3w2.T + b2
            h3ps = psm.tile([COUT, 1], f32, tag="mlp")
            nc.tensor.matmul(h3ps[:], wbt[0:64, C_A3W1:C_A3W1 + 64], gsum[:],
                             start=True, stop=True)
            h3r = sing.tile([COUT, 1], f32)
            nc.scalar.activation(out=h3r[:], in_=h3ps[:], func=AF.Relu,
                                 bias=wbt[0:64, C_A3B1:C_A3B1 + 1], scale=1.0 / P)
            b3ps = psm.tile([COUT, 1], f32, tag="mlp")
            nc.tensor.matmul(b3ps[:], wbt[0:64, C_A3W2:C_A3W2 + 64], h3r[:],
                             start=True, stop=True)
            bias3 = sing.tile([COUT, 1], f32)
            nc.scalar.activation(out=bias3[:], in_=b3ps[:], func=AF.Identity,
                                 bias=wbt[0:64, C_A3B2:C_A3B2 + 1], scale=1.0)

            # fold ratio into conv weights: wf_g = wl_g * rrep_g  (f16)
            wf = []
            for g, (r0, r1, o, u) in enumerate(GROUPS):
                parts = 128 if r1 is not None else 64
                t = sing.tile([parts, COUT], f16, tag=f"wf{g}")
                nc.vector.tensor_scalar(t[:], wbt[0:parts, C_WL + g * 64:
                                                 C_WL + (g + 1) * 64],
                                        rrep[g][0:parts, :], None, OP.mult)
                wf.append(t)

            # main pixel-chunk loop
            dmai = 0
            for k in range(NCH):
                col0 = k * CH + GUARD - GL  # XXL/ag column of q0 = k*CH - GL
                # attention replica tiles (DRAM->SBUF broadcast DMA)
                arts = []
                for g, (r0, r1, o, u) in enumerate(GROUPS):
                    s0 = col0 - o

                    def bcast_rows(r, nrow):
                        row = ag[r:r + 1, s0:s0 + WCH]
                        colap = list(row.ap)[-1]
                        if nrow == 1:
                            return bass.AP(tensor=row.tensor, offset=row.offset,
                                           ap=[[0, 64], colap])
                        return bass.AP(tensor=row.tensor, offset=row.offset,
                                       ap=[[WX, nrow], [0, 64], colap])

                    if g == 3:
                        # split tiles: TensorTensor needs equal input base
                        # partitions; both halves multiply base-0 x windows
                        a0 = arp.tile([64, WCH], f16, tag="ar3a")
                        a1 = arp.tile([64, WCH], f16, tag="ar3b")
                        dq(dmai).dma_start(out=a0[:], in_=bcast_rows(r0, 1))
                        dmai += 1
                        dq(dmai).dma_start(out=a1[:], in_=bcast_rows(r1, 1))
                        dmai += 1
                        arts.append((a0, a1))
                        continue
                    if r1 is None:
                        # lone tap reads the prebuilt full-width replica
                        arts.append(None)
                        continue
                    parts = 128
                    art = arp.tile([parts, WCH], f16, tag=f"ar{g}")
                    if g in AR_VIA_PE:
                        # PE broadcast: [2,128] selector x [2,cols] attention
                        # rows -> 64 replicas of each row, ScalarE evac-cast
                        for s5 in range(5):
                            cw = min(512, WCH - s5 * 512)
                            pb = psb.tile([128, 512], f32, tag="pb")
                            nc.tensor.matmul(
                                pb[:, 0:cw], sel[g],
                                atg[g][:, s0 + s5 * 512:s0 + s5 * 512 + cw],
                                start=True, stop=True)
                            nc.scalar.activation(
                                out=art[:, s5 * 512:s5 * 512 + cw],
                                in_=pb[:, 0:cw], func=AF.Copy)
                        arts.append(art)
                        continue
                    nrow = 2 if r1 is not None else 1
                    dq(dmai).dma_start(out=art[:], in_=bcast_rows(r0, nrow))
                    dmai += 1
                    arts.append(art)

                # R tiles: f16 tensor_tensor multiplies (2x mode on DVE;
                # the half-width single tap rides the idle GpSimd engine)
                rts = []
                for g, (r0, r1, o, u) in enumerate(GROUPS):
                    parts = 128 if r1 is not None else 64
                    rt = rp.tile([parts, WCH], f16, tag=f"r{g}")
                    if g < 3:
                        nc.vector.tensor_mul(rt[:], XXL[:, col0:col0 + WCH],
                                             arts[g][:])
                    elif g == 3:
                        nc.vector.tensor_mul(rt[0:64, :],
                                             XXL[0:64, col0:col0 + WCH],
                                             arts[g][0][:])
                        nc.vector.tensor_mul(rt[64:128, :],
                                             XXL[0:64, col0 + 128:col0 + 128 + WCH],
                                             arts[g][1][:])
                    else:
                        s0g = col0 - o
                        nc.gpsimd.tensor_mul(rt[:], XXL[0:64, col0:col0 + WCH],
                                             ar8F[:, s0g:s0g + WCH])
                    rts.append(rt)

                # matmuls: two 1024-pixel PSUM tiles per chunk, 512-col slices
                for half in range(2):
                    yps = psy.tile([COUT, 1024], f32)
                    for g, (r0, r1, o, u) in enumerate(GROUPS):
                        for s in range(2):
                            c0 = (o + GL) + half * 1024 + s * 512
                            nc.tensor.matmul(yps[:, s * 512:(s + 1) * 512],
                                             wf[g][:], rts[g][:, c0:c0 + 512],
                                             start=(g == 0), stop=(g == 4))
                    ysb = yp.tile([COUT, 1024], f16)
                    nc.scalar.activation(out=ysb[:], in_=yps[:], func=AF.Identity,
                                         bias=bias3[:], scale=1.0)
                    dq(dmai).dma_start(
                        out=y[:, k * CH + half * 1024: k * CH + (half + 1) * 1024],
                        in_=ysb[:])
                    dmai += 1
    nc.compile()
    return nc


def _host_prep(x, atw1, weight, a2_w1, a2_b1, a2_w2, a2_b2, a3_w1, a3_b1,
               a3_w2, a3_b2):
    """Build per-core input maps (host-side layout/dtype prep only)."""
    b = x.shape[0]
    f16 = np.float16

    # validity mask per permuted row: conv zero-padding baked into attention
    hh = np.arange(P) // WW
    ww = np.arange(P) % WW
    masks = np.empty((9, P), np.bool_)
    for r, t in enumerate(PERM):
        kh, kw = t // 3, t % 3
        masks[r] = ((hh + kh - 1 >= 0) & (hh + kh - 1 < HH) &
                    (ww + kw - 1 >= 0) & (ww + kw - 1 < WW))

    blob = np.zeros((128, BLOB_W), np.float32)
    for g, (r0, r1, o, u) in enumerate(GROUPS):
        t0 = PERM[r0]
        blob[0:64, C_WL + g * 64:C_WL + (g + 1) * 64] = \
            weight[:, :, t0 // 3, t0 % 3].T
        if r1 is not None:
            t1 = PERM[r1]
            blob[64:128, C_WL + g * 64:C_WL + (g + 1) * 64] = \
                weight[:, :, t1 // 3, t1 % 3].T
    blob[0:64, C_A2W1:C_A2W1 + 9] = a2_w1.T
    blob[0:9, C_A2B1] = a2_b1
    for g, (r0, r1, o, u) in enumerate(GROUPS):
        blob[0:9, C_A2W2R + g * 128:C_A2W2R + g * 128 + 64] = \
            a2_w2[PERM[r0]][:, None]
        blob[0:64, C_A2B2R + g] = a2_b2[PERM[r0]]
        if r1 is not None:
            blob[0:9, C_A2W2R + g * 128 + 64:C_A2W2R + (g + 1) * 128] = \
                a2_w2[PERM[r1]][:, None]
            blob[64:128, C_A2B2R + g] = a2_b2[PERM[r1]]
    blob[0:64, C_A3W1:C_A3W1 + 64] = a3_w1.T
    blob[0:64, C_A3B1] = a3_b1
    blob[0:64, C_A3W2:C_A3W2 + 64] = a3_w2.T
    blob[0:64, C_A3B2] = a3_b2

    selh = np.zeros((34, 128), np.float16)
    for j in range(2):
        selh[32 * j, 0:64] = 1.0
        selh[32 * j + 1, 64:128] = 1.0

    in_maps = []
    for i in range(b):
        xr = x[i].reshape(CIN, P)
        xgv = np.zeros((CIN, WX), f16)
        xgv[:, GUARD:GUARD + P] = xr.astype(f16)
        at = atw1[i].reshape(9, P)[PERM]
        agv = np.zeros((9, WX), f16)
        agv[:, GUARD:GUARD + P] = np.where(masks, at, 0.0).astype(f16)
        m = {"xg": xgv, "ag": agv, "wb": blob}
        if AR_VIA_PE:
            m["sg"] = selh
        in_maps.append(m)
    return in_maps


def kernel(**inputs):
    x = np.asarray(inputs["x"], np.float32)
    in_maps = _host_prep(
        x, np.asarray(inputs["atw1"], np.float32),
        np.asarray(inputs["weight"], np.float32),
        np.asarray(inputs["a2_w1"], np.float32),
        np.asarray(inputs["a2_b1"], np.float32),
        np.asarray(inputs["a2_w2"], np.float32),
        np.asarray(inputs["a2_b2"], np.float32),
        np.asarray(inputs["a3_w1"], np.float32),
        np.asarray(inputs["a3_b1"], np.float32),
        np.asarray(inputs["a3_w2"], np.float32),
        np.asarray(inputs["a3_b2"], np.float32),
    )
    if "nc" not in _CACHE:
        _CACHE["nc"] = _build_nc()
    nc = _CACHE["nc"]
    res = run_bass_kernel_spmd(nc, in_maps, core_ids=list(range(8)))
    out = np.stack([res.results[i]["y"].reshape(COUT, HH, WW)
                    for i in range(len(in_maps))])
    return out.astype(np.float32)


# revision 25
# speedup vs baseline: 1.7311x; 1.0933x over previous
"""Trainium2 Bass kernel for nn_AFM (attention-modulated 3x3 conv).

Math (per batch):
    ratio = MLP_a2(mean_hw(x))                       # [9]
    bias3 = MLP_a3(mean_hw(x))                       # [64]
    y[m,p] = sum_{c,t} W[m,c,t] * x[c, p+delta_t] * (atw1[t,p]*ratio[t]) + bias3[m]

Strategy: data-parallel over batch (8 cores, 1 batch each, no collectives).
Per core, fp16 compute:
  - attention (with conv zero-pad validity baked in as zeros, guard columns)
    ships as an f16 input `ag`; x ships as guarded f16 `xg`.
  - ratio is folded into the conv weights on device (so atw1 is used raw).
  - taps are processed in pairs sharing one 128-row contraction:
      R_pair[(c,t), q] = x[c, q+u_t] * ag[t, q - o_pair]
    built by DVE f16 tensor_tensor (2x mode), with the per-pair attention
    replica tiles produced by DRAM->SBUF broadcast DMA (partition step 0),
    spread round-robin over issue engines to use multiple DMA queues.
  - 5 PSUM-accumulated f16 matmuls per 512-pixel tile compute y directly;
    ScalarE evacuates PSUM with the a3-bias add fused.
"""

import numpy as np
from contextlib import ExitStack

import concourse.bass as bass
import concourse.tile as tile
from concourse import bacc, mybir
from concourse.bass_utils import run_bass_kernel_spmd

# permuted tap order: rows are taps [0,2, 3,5, 6,8, 1,4, 7] so that each
# matmul group's two taps sit on adjacent rows of `ag`
PERM = [0, 2, 3, 5, 6, 8, 1, 4, 7]
# groups: (row0, row1|None, o = rhs pixel offset, u = upper-half x shift)
GROUPS = [
    (0, 1, -129, 2),
    (2, 3, -1, 2),
    (4, 5, 127, 2),
    (6, 7, -128, 128),
    (8, None, 128, 0),
]
HH = 128
WW = 128
P = HH * WW           # 16384 pixels
CIN = 64
COUT = 64
GUARD = 264           # zero guard columns on xg/ag (>= 132 + 129)
GL = 132              # per-chunk halo for rhs offsets (|o| <= 129)
CH = 2048             # pixels per chunk
NCH = P // CH
WCH = CH + 2 * GL     # 2312 columns per R/ar chunk tile
WX = P + 2 * GUARD    # 16912
AR_VIA_PE = ()    # pair-groups whose attention replicas come from a
                      # PE broadcast-matmul + ScalarE evac instead of DMA

# packed small-weight blob layout (f32, [128, BLOB_W]); columns:
#   wl: 5 groups x 64      -> 0..320     (rows 0..127)
#   a2w1T [64,9]           -> 320..329   (rows 0..63)
#   a2b1 [9,1]             -> 329..330   (rows 0..8)
#   a2w2r [9,640]          -> 330..970   (rows 0..8)
#   a2b2r [128,5]          -> 970..975
#   a3w1T [64,64]          -> 975..1039  (rows 0..63)
#   a3b1 [64,1]            -> 1039..1040
#   a3w2T [64,64]          -> 1040..1104
#   a3b2 [64,1]            -> 1104..1105
BLOB_W = 1105
C_WL, C_A2W1, C_A2B1, C_A2W2R, C_A2B2R = 0, 320, 329, 330, 970
C_A3W1, C_A3B1, C_A3W2, C_A3B2 = 975, 1039, 1040, 1104

_CACHE = {}


def _build_nc():
    f32, f16 = mybir.dt.float32, mybir.dt.float16
    AF = mybir.ActivationFunctionType
    OP = mybir.AluOpType

    nc = bacc.Bacc("TRN2", target_bir_lowering=False, debug=False,
                   enable_asserts=True, num_devices=8)
    xg = nc.dram_tensor("xg", [CIN, WX], f16, kind="ExternalInput").ap()
    ag = nc.dram_tensor("ag", [9, WX], f16, kind="ExternalInput").ap()
    wb = nc.dram_tensor("wb", [128, BLOB_W], f32, kind="ExternalInput").ap()
    if AR_VIA_PE:
        sg = nc.dram_tensor("sg", [34, 128], mybir.dt.float16,
                            kind="ExternalInput").ap()
    y = nc.dram_tensor("y", [COUT, P], f16, kind="ExternalOutput").ap()

    # round-robin DMA issue engines (separate queues)
    def dq(i):
        return [nc.sync, nc.scalar][i % 2]

    with tile.TileContext(nc) as tc:
        with ExitStack() as ctx:
            sing = ctx.enter_context(tc.tile_pool(name="sing", bufs=1))
            arp = ctx.enter_context(tc.tile_pool(name="arp", bufs=2))
            rp = ctx.enter_context(tc.tile_pool(name="rp", bufs=3))
            yp = ctx.enter_context(tc.tile_pool(name="yp", bufs=2))
            trp = ctx.enter_context(tc.tile_pool(name="trp", bufs=2))
            psy = ctx.enter_context(tc.tile_pool(name="psy", bufs=2, space="PSUM"))
            psm = ctx.enter_context(tc.tile_pool(name="psm", bufs=2, space="PSUM"))
            psb = ctx.enter_context(tc.tile_pool(name="psb", bufs=2, space="PSUM"))

            # small-weight blob
            wbt = sing.tile([128, BLOB_W], f32)
            nc.sync.dma_start(out=wbt, in_=wb)

            # attention rows in SBUF (PE broadcast source): one tile, each
            # PE-pair's 2 rows at a legal matmul base partition (0, 32)
            atg, sel = {}, {}
            if AR_VIA_PE:
                atgt = sing.tile([34, WX], f16)
                selt = sing.tile([34, 128], f16)
                nc.sync.dma_start(out=selt, in_=sg)
                for j, g in enumerate(AR_VIA_PE):
                    r0 = GROUPS[g][0]
                    nc.scalar.dma_start(out=atgt[32 * j:32 * j + 2, :],
                                        in_=ag[r0:r0 + 2, :])
                    atg[g] = atgt[32 * j:32 * j + 2, :]
                    sel[g] = selt[32 * j:32 * j + 2, :]

            # x (f16, guarded) -> lower half of XXL, loaded in chunks; the
            # g-mean accumulation runs per chunk, alternating ScalarE (f16
            # copy with accum_out) and VectorE (reduce); the +2-shifted upper
            # half is built in per-chunk pieces so early chunks unblock fast
            XXL = sing.tile([128, WX], f16)
            gparts = sing.tile([CIN, NCH], f32)
            nc.scalar.dma_start(out=XXL[0:64, 0:GUARD], in_=xg[:, 0:GUARD])
            nc.gpsimd.dma_start(out=XXL[0:64, GUARD + P:], in_=xg[:, GUARD + P:])
            for k in range(NCH):
                c0 = GUARD + k * CH
                dq(k).dma_start(out=XXL[0:64, c0:c0 + CH], in_=xg[:, c0:c0 + CH])
                if k % 2 == 0:
                    trash = trp.tile([CIN, CH], f16, tag="trash")
                    nc.scalar.activation(out=trash[:], in_=XXL[0:64, c0:c0 + CH],
                                         func=AF.Copy,
                                         accum_out=gparts[:, k:k + 1])
                else:
                    nc.vector.tensor_reduce(gparts[:, k:k + 1],
                                            XXL[0:64, c0:c0 + CH],
                                            axis=mybir.AxisListType.X, op=OP.add)
                if k > 0:
                    w0 = (k - 1) * CH + GUARD - GL
                    nc.vector.tensor_copy(out=XXL[64:128, w0:w0 + WCH],
                                          in_=XXL[0:64, w0 + 2:w0 + 2 + WCH])
            w0 = (NCH - 1) * CH + GUARD - GL
            nc.vector.tensor_copy(out=XXL[64:128, w0:w0 + WCH],
                                  in_=XXL[0:64, w0 + 2:w0 + 2 + WCH])

            # full-width replica of the lone tap's attention row, built once
            # (emitted after the x loads so they win queue priority)
            r8 = GROUPS[4][0]
            row8 = ag[r8:r8 + 1, :]
            ar8F = sing.tile([64, WX], f16)
            nc.sync.dma_start(out=ar8F[:], in_=bass.AP(
                tensor=row8.tensor, offset=row8.offset,
                ap=[[0, 64], list(row8.ap)[-1]]))

            gsum = sing.tile([CIN, 1], f32)
            nc.vector.tensor_reduce(gsum[:], gparts[:], axis=mybir.AxisListType.X,
                                    op=OP.add)

            # attention2: h1 = relu(g @ a2w1.T / P + b1); ratio replicated per
            # group via host-widened a2w2 columns
            h1ps = psm.tile([9, 1], f32, tag="mlp")
            nc.tensor.matmul(h1ps[:], wbt[0:64, C_A2W1:C_A2W1 + 9], gsum[:],
                             start=True, stop=True)
            h1r = sing.tile([9, 1], f32)
            nc.scalar.activation(out=h1r[:], in_=h1ps[:], func=AF.Relu,
                                 bias=wbt[0:9, C_A2B1:C_A2B1 + 1], scale=1.0 / P)
            rrep = []
            for g in range(5):
                rps = psm.tile([128, 1], f32, tag="mlp")
                nc.tensor.matmul(
                    rps[:], wbt[0:9, C_A2W2R + g * 128:C_A2W2R + (g + 1) * 128],
                    h1r[:], start=True, stop=True)
                rr = sing.tile([128, 1], f32, tag=f"rr{g}")
                nc.scalar.activation(out=rr[:], in_=rps[:], func=AF.Identity,
                                     bias=wbt[:, C_A2B2R + g:C_A2B2R + g + 1],
                                     scale=1.0)
                rrep.append(rr)

            # attention3 bias: bias3 = relu(g @ a3w1.T / P + b1) @ a3w2.T + b2
            h3ps = psm.tile([COUT, 1], f32, tag="mlp")
            nc.tensor.matmul(h3ps[:], wbt[0:64, C_A3W1:C_A3W1 + 64], gsum[:],
                             start=True, stop=True)
            h3r = sing.tile([COUT, 1], f32)
            nc.scalar.activation(out=h3r[:], in_=h3ps[:], func=AF.Relu,
                                 bias=wbt[0:64, C_A3B1:C_A3B1 + 1], scale=1.0 / P)
            b3ps = psm.tile([COUT, 1], f32, tag="mlp")
            nc.tensor.matmul(b3ps[:], wbt[0:64, C_A3W2:C_A3W2 + 64], h3r[:],
                             start=True, stop=True)
            bias3 = sing.tile([COUT, 1], f32)
            nc.scalar.activation(out=bias3[:], in_=b3ps[:], func=AF.Identity,
                                 bias=wbt[0:64, C_A3B2:C_A3B2 + 1], scale=1.0)

            # fold ratio into conv weights: wf_g = wl_g * rrep_g  (f16)
            wf = []
            for g, (r0, r1, o, u) in enumerate(GROUPS):
                parts = 128 if r1 is not None else 64
                t = sing.tile([parts, COUT], f16, tag=f"wf{g}")
                nc.vector.tensor_scalar(t[:], wbt[0:parts, C_WL + g * 64:
                                                 C_WL + (g + 1) * 64],
                                        rrep[g][0:parts, :], None, OP.mult)
                wf.append(t)

            # main pixel-chunk loop, seam-split tiling: R chunk k covers
            # pixels q in [qlo_k, (k+1)*CH) where qlo_0 = -GL (left halo
            # folded into a widened chunk 0); matmul rhs reads split at chunk
            # seams so no halo columns are ever recomputed or re-broadcast.
            # Right-edge reads past P are dropped: their attention is masked
            # to zero (out-of-image taps), so the products are exactly zero.
            def chunk_w(k):
                return CH + (GL if k == 0 else 0)

            def rloc(q):
                # (chunk, column) of pixel q within the R tiles
                if q < CH:
                    return 0, q + GL
                return q // CH, q % CH

            rtiles = []
            dmai = 0

            def emit_matmuls(m):
                for half in range(2):
                    yps = psy.tile([COUT, 1024], f32)
                    started = [False, False]
                    for g, (r0, r1, o, u) in enumerate(GROUPS):
                        last = g == len(GROUPS) - 1
                        for s in range(2):
                            q0 = m * CH + half * 1024 + s * 512 + o
                            k1, c1 = rloc(q0)
                            bnd = (k1 + 1) * CH
                            pcol = s * 512
                            if q0 + 512 <= bnd:
                                nc.tensor.matmul(
                                    yps[:, pcol:pcol + 512], wf[g][:],
                                    rtiles[k1][g][:, c1:c1 + 512],
                                    start=not started[s], stop=last)
                                started[s] = True
                            else:
                                w1 = bnd - q0
                                tail = k1 + 1 <= NCH - 1
                                nc.tensor.matmul(
                                    yps[:, pcol:pcol + w1], wf[g][:],
                                    rtiles[k1][g][:, c1:c1 + w1],
                                    start=not started[s],
                                    stop=last and not tail)
                                started[s] = True
                                if tail:
                                    nc.tensor.matmul(
                                        yps[:, pcol + w1:pcol + 512], wf[g][:],
                                        rtiles[k1 + 1][g][:, 0:512 - w1],
                                        start=False, stop=last)
                    ysb = yp.tile([COUT, 1024], f16)
                    nc.scalar.activation(out=ysb[:], in_=yps[:], func=AF.Identity,
                                         bias=bias3[:], scale=1.0)
                    nonlocal_dma = dq(m + half)
                    nonlocal_dma.dma_start(
                        out=y[:, m * CH + half * 1024: m * CH + (half + 1) * 1024],
                        in_=ysb[:])

            for k in range(NCH):
                wk = chunk_w(k)
                qlo = k * CH - (GL if k == 0 else 0)
                col0 = qlo + GUARD          # XXL column of the chunk start

                # attention replica tiles (DRAM->SBUF broadcast DMA)
                arts = []
                for g, (r0, r1, o, u) in enumerate(GROUPS):
                    s0 = qlo - o + GUARD

                    def bcast_rows(r, nrow, w):
                        row = ag[r:r + 1, s0:s0 + w]
                        colap = list(row.ap)[-1]
                        if nrow == 1:
                            return bass.AP(tensor=row.tensor, offset=row.offset,
                                           ap=[[0, 64], colap])
                        return bass.AP(tensor=row.tensor, offset=row.offset,
                                       ap=[[0, 64], [WX, nrow], colap])

                    if g == 3:
                        # one DMA: both halves column-concatenated (base
                        # partition 0 for both TT reads)
                        a01 = arp.tile([64, 2 * wk], f16, tag="ar3")
                        dq(dmai).dma_start(out=a01[:], in_=bcast_rows(r0, 2, wk))
                        dmai += 1
                        arts.append(a01)
                        continue
                    if r1 is None:
                        arts.append(None)  # lone tap uses prebuilt ar8F
                        continue
                    art = arp.tile([128, wk], f16, tag=f"ar{g}")
                    row = ag[r0:r0 + 1, s0:s0 + wk]
                    src = bass.AP(tensor=row.tensor, offset=row.offset,
                                  ap=[[WX, 2], [0, 64], list(row.ap)[-1]])
                    dq(dmai).dma_start(out=art[:], in_=src)
                    dmai += 1
                    arts.append(art)

                # R tiles: f16 tensor_tensor multiplies (2x mode on DVE;
                # the lone half-width tap rides the GpSimd engine)
                rts = []
                for g, (r0, r1, o, u) in enumerate(GROUPS):
                    parts = 128 if r1 is not None else 64
                    rt = rp.tile([parts, wk], f16, tag=f"r{g}")
                    if g < 3:
                        nc.vector.tensor_mul(rt[:], XXL[:, col0:col0 + wk],
                                             arts[g][:])
                    elif g == 3:
                        nc.vector.tensor_mul(rt[0:64, :],
                                             XXL[0:64, col0:col0 + wk],
                                             arts[g][:, 0:wk])
                        nc.vector.tensor_mul(rt[64:128, :],
                                             XXL[0:64, col0 + 128:col0 + 128 + wk],
                                             arts[g][:, wk:2 * wk])
                    else:
                        s0g = qlo - o + GUARD
                        nc.gpsimd.tensor_mul(rt[:], XXL[0:64, col0:col0 + wk],
                                             ar8F[:, s0g:s0g + wk])
                    rts.append(rt)
                rtiles.append(rts)

                if k >= 1:
                    emit_matmuls(k - 1)
            emit_matmuls(NCH - 1)
    nc.compile()
    return nc


def _host_prep(x, atw1, weight, a2_w1, a2_b1, a2_w2, a2_b2, a3_w1, a3_b1,
               a3_w2, a3_b2):
    """Build per-core input maps (host-side layout/dtype prep only)."""
    b = x.shape[0]
    f16 = np.float16

    # validity mask per permuted row: conv zero-padding baked into attention
    hh = np.arange(P) // WW
    ww = np.arange(P) % WW
    masks = np.empty((9, P), np.bool_)
    for r, t in enumerate(PERM):
        kh, kw = t // 3, t % 3
        masks[r] = ((hh + kh - 1 >= 0) & (hh + kh - 1 < HH) &
                    (ww + kw - 1 >= 0) & (ww + kw - 1 < WW))

    blob = np.zeros((128, BLOB_W), np.float32)
    for g, (r0, r1, o, u) in enumerate(GROUPS):
        t0 = PERM[r0]
        blob[0:64, C_WL + g * 64:C_WL + (g + 1) * 64] = \
            weight[:, :, t0 // 3, t0 % 3].T
        if r1 is not None:
            t1 = PERM[r1]
            blob[64:128, C_WL + g * 64:C_WL + (g + 1) * 64] = \
                weight[:, :, t1 // 3, t1 % 3].T
    blob[0:64, C_A2W1:C_A2W1 + 9] = a2_w1.T
    blob[0:9, C_A2B1] = a2_b1
    for g, (r0, r1, o, u) in enumerate(GROUPS):
        blob[0:9, C_A2W2R + g * 128:C_A2W2R + g * 128 + 64] = \
            a2_w2[PERM[r0]][:, None]
        blob[0:64, C_A2B2R + g] = a2_b2[PERM[r0]]
        if r1 is not None:
            blob[0:9, C_A2W2R + g * 128 + 64:C_A2W2R + (g + 1) * 128] = \
                a2_w2[PERM[r1]][:, None]
            blob[64:128, C_A2B2R + g] = a2_b2[PERM[r1]]
    blob[0:64, C_A3W1:C_A3W1 + 64] = a3_w1.T
    blob[0:64, C_A3B1] = a3_b1
    blob[0:64, C_A3W2:C_A3W2 + 64] = a3_w2.T
    blob[0:64, C_A3B2] = a3_b2

    selh = np.zeros((34, 128), np.float16)
    for j in range(2):
        selh[32 * j, 0:64] = 1.0
        selh[32 * j + 1, 64:128] = 1.0

    in_maps = []
    for i in range(b):
        xr = x[i].reshape(CIN, P)
        xgv = np.zeros((CIN, WX), f16)
        xgv[:, GUARD:GUARD + P] = xr.astype(f16)
        at = atw1[i].reshape(9, P)[PERM]
        agv = np.zeros((9, WX), f16)
        agv[:, GUARD:GUARD + P] = np.where(masks, at, 0.0).astype(f16)
        m = {"xg": xgv, "ag": agv, "wb": blob}
        if AR_VIA_PE:
            m["sg"] = selh
        in_maps.append(m)
    return in_maps


def kernel(**inputs):
    x = np.asarray(inputs["x"], np.float32)
    in_maps = _host_prep(
        x, np.asarray(inputs["atw1"], np.float32),
        np.asarray(inputs["weight"], np.float32),
        np.asarray(inputs["a2_w1"], np.float32),
        np.asarray(inputs["a2_b1"], np.float32),
        np.asarray(inputs["a2_w2"], np.float32),
        np.asarray(inputs["a2_b2"], np.float32),
        np.asarray(inputs["a3_w1"], np.float32),
        np.asarray(inputs["a3_b1"], np.float32),
        np.asarray(inputs["a3_w2"], np.float32),
        np.asarray(inputs["a3_b2"], np.float32),
    )
    if "nc" not in _CACHE:
        _CACHE["nc"] = _build_nc()
    nc = _CACHE["nc"]
    res = run_bass_kernel_spmd(nc, in_maps, core_ids=list(range(8)))
    out = np.stack([res.results[i]["y"].reshape(COUT, HH, WW)
                    for i in range(len(in_maps))])
    return out.astype(np.float32)
